# revision 25
# baseline (speedup 1.0000x reference)
"""Trainium2 Bass kernel for MultiHeadLatentAttention (MLA), 8-core SPMD.

Sharding: core c = (batch b=c//4, head-group g=c%4 of 4 heads).
Each core computes the full latent down-projections for its batch
(replicated across the 4 cores of that batch), head-sharded
up-projections + RoPE + causal attention, and a partial o-projection
(its heads' rows of Wo). The host sums the 4 partial outputs per batch.

Shapes (fixed): B=2, S=2048, H=2048, L=256, nh=16, hd=128, rd=64.

Device layouts (features on partitions):
  kv_dT, q_dT [L=256 -> 2x128, s]
  k_rT        [2x128, s] chunk0 = all heads' rope-lo rows (4x32),
                         chunk1 = all heads' rope-hi rows; rotated in place
  qT_h, kT_h  [128 (64 content + 64 rope), s] per head
  v_all       [128 (s%128), s//128, 4 heads * 128]  (natural v)
  yT_all      [128 (hd), 4 heads, s]

Attention is computed in scores-transposed orientation S^T[k, q] so the
probabilities feed the AV matmul directly (lhsT = v block, rhs = expS).
Softmax skips the max-subtraction (scores are tiny here: |s| < ~2). The
denominator is accumulated per k-block by an all-ones matmul into PSUM,
which also broadcasts it across partitions for the final normalize.
"""

import sys
import os

sys.path.insert(0, "/opt/trn_rl_repo")

import numpy as np

B = 2
S = 2048
H = 2048
L = 256          # latent dim (2 chunks of 128)
NH = 16          # total heads
HPC = 4          # heads per core
HD = 128         # head dim
RD = 64          # rope / content half-width
ROPE_BASE = 10000.0
SCALE = float(HD) ** -0.5

SB = 512         # s-block for projections / q-blocks in attention
KB = 128         # k-block in attention
NKC = H // 128   # 16 contraction chunks over H
NLC = L // 128   # 2 contraction chunks over L

# Matmul input dtype: "f32" (exact, 4 cyc/row) or "f32r" (1 cyc/row, ~tf32).
MM_DTYPE = os.environ.get("MLA_MM_DTYPE", "f32")


def build_nc(s=S, mm_dtype=MM_DTYPE):
    """Build the Bass module for one core. `s` can be shrunk (multiple of 512)
    for simulator testing."""
    from concourse import bacc
    import concourse.bass as bass
    import concourse.mybir as mybir
    import concourse.tile as tile
    from concourse.masks import make_identity
    from contextlib import ExitStack

    f32 = mybir.dt.float32
    f32r = mybir.dt.float32r

    # dtype for every tile/DRAM tensor that feeds a matmul: the BIR
    # verifier requires producers of fp32r-matmul operands to WRITE fp32r.
    mdt = f32r if mm_dtype == "f32r" else f32

    nsb = s // SB        # s-blocks
    nsc = s // 128       # 128-row s-chunks

    nc = bacc.Bacc(None, target_bir_lowering=False)

    hs = nc.dram_tensor("hs", [s, H], mdt, kind="ExternalInput")
    w_down = nc.dram_tensor("w_down", [H, 512], mdt, kind="ExternalInput")
    w_rk = nc.dram_tensor("w_rk", [H, HPC * RD], mdt, kind="ExternalInput")
    w_qc = nc.dram_tensor("w_qc", [L, HPC * RD], mdt, kind="ExternalInput")
    w_qr = nc.dram_tensor("w_qr", [L, HPC * RD], mdt, kind="ExternalInput")
    w_ku = nc.dram_tensor("w_ku", [L, HPC * RD], mdt, kind="ExternalInput")
    w_vu = nc.dram_tensor("w_vu", [L, HPC * HD], mdt, kind="ExternalInput")
    w_o = nc.dram_tensor("w_o", [HPC * HD, H], mdt, kind="ExternalInput")
    # cos/sin halves replicated across all four 32-partition quadrants so any
    # 32-row operand can pair with a table slice at the SAME base partition
    # (walrus: both-SBUF tensor_tensor inputs must share base partition).
    rope_cc = nc.dram_tensor("rope_cc", [128, s], f32, kind="ExternalInput")
    rope_ss = nc.dram_tensor("rope_ss", [128, s], f32, kind="ExternalInput")
    out = nc.dram_tensor("out", [s, H], f32, kind="ExternalOutput")

    Exp = mybir.ActivationFunctionType.Exp
    is_ge = mybir.AluOpType.is_ge

    with ExitStack() as top:
        tc = top.enter_context(tile.TileContext(nc))

        # ---- persistent small pools -------------------------------------
        const_pool = top.enter_context(tc.tile_pool(name="const", bufs=1))
        ident = const_pool.tile([128, 128], mdt, tag="ident")
        if mdt == f32:
            make_identity(nc, ident)
        else:
            ident_f32 = const_pool.tile([128, 128], f32, tag="ident_f32")
            make_identity(nc, ident_f32)
            nc.vector.tensor_copy(ident[:], ident_f32[:])
        ones128 = const_pool.tile([128, 128], mdt, tag="ones")
        if mdt == f32:
            nc.gpsimd.memset(ones128[:], 1.0)
        else:
            ones_f32 = const_pool.tile([128, 128], f32, tag="ones_f32")
            nc.gpsimd.memset(ones_f32[:], 1.0)
            nc.vector.tensor_copy(ones128[:], ones_f32[:])
        cc_t = const_pool.tile([128, s], f32, tag="ropec")
        ss_t = const_pool.tile([128, s], f32, tag="ropes")

        wsmall = top.enter_context(tc.tile_pool(name="wsmall", bufs=1))
        w_qc_t = wsmall.tile([128, NLC, HPC * RD], mdt, tag="wqc")
        w_qr_t = wsmall.tile([128, NLC, HPC * RD], mdt, tag="wqr")
        w_ku_t = wsmall.tile([128, NLC, HPC * RD], mdt, tag="wku")
        w_vu_t = wsmall.tile([128, NLC, HPC * HD], mdt, tag="wvu")

        # ---- latent / rope-k tensors (persist through attention) --------
        lat_pool = top.enter_context(tc.tile_pool(name="lat", bufs=1))
        kv_dT = lat_pool.tile([128, NLC, s], mdt, tag="kvd")   # [L, s]
        q_dT = lat_pool.tile([128, NLC, s], mdt, tag="qd")     # [L, s]
        k_rT = lat_pool.tile([128, NLC, s], mdt, tag="krt")    # rope k rows

        # ================= PHASE A: transpose hs + down/rope projections =
        with ExitStack() as pa:
            srcp = pa.enter_context(tc.tile_pool(name="src", bufs=4))
            hstp = pa.enter_context(tc.tile_pool(name="hst", bufs=1))
            wdp = pa.enter_context(tc.tile_pool(name="wdown", bufs=1))
            wrkp = pa.enter_context(tc.tile_pool(name="wrk", bufs=1))
            pst = pa.enter_context(tc.tile_pool(name="pst", bufs=2, space="PSUM"))
            psa = pa.enter_context(tc.tile_pool(name="psa", bufs=1, space="PSUM"))

            # per-chunk weight DMAs so the first matmuls don't wait for the
            # whole 4MB tensor
            w_down_t = wdp.tile([128, NKC, 512], mdt, tag="wd")
            w_rk_t = wrkp.tile([128, NKC, HPC * RD], mdt, tag="wr")
            wd_r = w_down.rearrange("(ko p) m -> p ko m", p=128)
            wr_r = w_rk.rearrange("(ko p) m -> p ko m", p=128)
            for kc in range(NKC):
                nc.sync.dma_start(w_down_t[:, kc, :], wd_r[:, kc, :])
                nc.sync.dma_start(w_rk_t[:, kc, :], wr_r[:, kc, :])

            names = ("kv0", "kv1", "q0", "q1", "kr0", "kr1")
            for sb in range(nsb):
                hsT = hstp.tile([128, NKC, SB], mdt, tag="hsT")
                srcs = []
                for sc in range(4):
                    srct = srcp.tile([128, H], mdt, tag="src")
                    nc.sync.dma_start(
                        srct[:],
                        hs[sb * SB + sc * 128: sb * SB + (sc + 1) * 128, :])
                    srcs.append(srct)

                # 6 accumulation banks for this s-block
                pb = {n: psa.tile([128, SB], f32, tag=f"psa_{n}",
                                  name=f"psa_{n}_{sb}") for n in names}

                for kc in range(NKC):
                    pt = pst.tile([128, SB], f32, tag="pt")
                    for sc in range(4):
                        nc.tensor.transpose(
                            pt[:, sc * 128:(sc + 1) * 128],
                            srcs[sc][:, kc * 128:(kc + 1) * 128],
                            ident[:])
                    nc.scalar.copy(hsT[:, kc, :], pt[:])
                    rhs = hsT[:, kc, :]
                    st = kc == 0
                    sp = kc == NKC - 1
                    nc.tensor.matmul(pb["kv0"][:], w_down_t[:, kc, 0:128],
                                     rhs, start=st, stop=sp)
                    nc.tensor.matmul(pb["kv1"][:], w_down_t[:, kc, 128:256],
                                     rhs, start=st, stop=sp)
                    nc.tensor.matmul(pb["q0"][:], w_down_t[:, kc, 256:384],
                                     rhs, start=st, stop=sp)
                    nc.tensor.matmul(pb["q1"][:], w_down_t[:, kc, 384:512],
                                     rhs, start=st, stop=sp)
                    nc.tensor.matmul(pb["kr0"][:], w_rk_t[:, kc, 0:128],
                                     rhs, start=st, stop=sp)
                    nc.tensor.matmul(pb["kr1"][:], w_rk_t[:, kc, 128:256],
                                     rhs, start=st, stop=sp)

                sbs = slice(sb * SB, (sb + 1) * SB)
                nc.scalar.copy(kv_dT[:, 0, sbs], pb["kv0"][:])
                nc.scalar.copy(kv_dT[:, 1, sbs], pb["kv1"][:])
                nc.scalar.copy(q_dT[:, 0, sbs], pb["q0"][:])
                nc.scalar.copy(q_dT[:, 1, sbs], pb["q1"][:])
                nc.scalar.copy(k_rT[:, 0, sbs], pb["kr0"][:])
                nc.scalar.copy(k_rT[:, 1, sbs], pb["kr1"][:])

        # late small-weight / rope-table loads: queued behind phase A's DMAs
        nc.sync.dma_start(cc_t[:], rope_cc[:])
        nc.sync.dma_start(ss_t[:], rope_ss[:])
        nc.sync.dma_start(
            w_qc_t[:], w_qc.rearrange("(ko p) m -> p ko m", p=128))
        nc.sync.dma_start(
            w_qr_t[:], w_qr.rearrange("(ko p) m -> p ko m", p=128))
        nc.sync.dma_start(
            w_ku_t[:], w_ku.rearrange("(ko p) m -> p ko m", p=128))
        nc.sync.dma_start(
            w_vu_t[:], w_vu.rearrange("(ko p) m -> p ko m", p=128))

        # ---- rotate k_rT in place (RoPE on all 4 heads at once) ---------
        # chunk0 rows = all heads' lo dims, chunk1 = hi dims:
        #   lo' = lo*cos - hi*sin ;  hi' = hi*cos + lo*sin
        with tc.tile_pool(name="rkr", bufs=2) as rkr:
            for sb in range(nsb):
                sbs = slice(sb * SB, (sb + 1) * SB)
                r1 = rkr.tile([128, SB], f32, tag="r1")
                r2 = rkr.tile([128, SB], f32, tag="r2")
                r3 = rkr.tile([128, SB], f32, tag="r3")
                r4 = rkr.tile([128, SB], f32, tag="r4")
                lo = k_rT[:, 0, sbs].bitcast(f32)
                hi = k_rT[:, 1, sbs].bitcast(f32)
                nc.vector.tensor_mul(r1[:], lo, cc_t[:, sbs])
                nc.vector.tensor_mul(r2[:], hi, ss_t[:, sbs])
                nc.vector.tensor_mul(r3[:], hi, cc_t[:, sbs])
                nc.vector.tensor_mul(r4[:], lo, ss_t[:, sbs])
                nc.vector.tensor_sub(k_rT[:, 0, sbs], r1[:], r2[:])
                nc.vector.tensor_add(k_rT[:, 1, sbs], r3[:], r4[:])

        # ================= PHASES B-C: v, per-head q/k + attention =======
        vp = top.enter_context(tc.tile_pool(name="vp", bufs=1))
        yp = top.enter_context(tc.tile_pool(name="yp", bufs=1))
        v_all = vp.tile([128, nsc, HPC * HD], mdt, tag="vall")
        yT_all = yp.tile([128, HPC, s], mdt, tag="yT")

        with ExitStack() as pbc:
            qkp = pbc.enter_context(tc.tile_pool(name="qkp", bufs=2))
            esp = pbc.enter_context(tc.tile_pool(name="esp", bufs=4))
            recp = pbc.enter_context(tc.tile_pool(name="recp", bufs=1))
            rtmp = pbc.enter_context(tc.tile_pool(name="rtmp", bufs=1))

            # ---- PHASE B: v (natural layout), all 4 heads ----
            # scoped psum pool, closed before phase C's psum pools open
            with tc.tile_pool(name="ps_v", bufs=2, space="PSUM") as ps_v:
                for sc in range(nsc):
                    pv = ps_v.tile([128, HPC * HD], f32, tag="pv")
                    for lc in range(NLC):
                        nc.tensor.matmul(
                            pv[:], kv_dT[:, lc, sc * 128:(sc + 1) * 128],
                            w_vu_t[:, lc, :],
                            start=(lc == 0), stop=(lc == NLC - 1))
                    nc.scalar.copy(v_all[:, sc, :], pv[:])

            ps_qk = pbc.enter_context(
                tc.tile_pool(name="ps_qk", bufs=1, space="PSUM"))
            ps_s = pbc.enter_context(
                tc.tile_pool(name="ps_s", bufs=3, space="PSUM"))
            ps_y = pbc.enter_context(
                tc.tile_pool(name="ps_y", bufs=2, space="PSUM"))
            ps_b = pbc.enter_context(
                tc.tile_pool(name="ps_b", bufs=2, space="PSUM"))

            for hp in range(2):            # head pairs
                h0, h1 = 2 * hp, 2 * hp + 1
                qT = {h: qkp.tile([128, s], mdt, tag="qT", name=f"qT_{h}")
                      for h in (h0, h1)}
                kT = {h: qkp.tile([128, s], mdt, tag="kT", name=f"kT_{h}")
                      for h in (h0, h1)}

                # ---- C1: projections + rope for both heads of the pair --
                for sb in range(nsb):
                    sbs = slice(sb * SB, (sb + 1) * SB)
                    # k content for both heads in one [128, SB] psum
                    pk = ps_qk.tile([128, SB], f32, tag="pqk")
                    for lc in range(NLC):
                        nc.tensor.matmul(
                            pk[:],
                            w_ku_t[:, lc, hp * 128:(hp + 1) * 128],
                            kv_dT[:, lc, sbs],
                            start=(lc == 0), stop=(lc == NLC - 1))
                    nc.vector.tensor_copy(kT[h0][0:64, sbs], pk[0:64, :])
                    nc.vector.tensor_copy(kT[h1][0:32, sbs], pk[64:96, :])
                    nc.vector.tensor_copy(kT[h1][32:64, sbs], pk[96:128, :])
                    # k rope: scatter pre-rotated k_rT rows (gpsimd, idle)
                    for h in (h0, h1):
                        rb = slice(32 * h, 32 * h + 32)
                        nc.gpsimd.tensor_copy(kT[h][64:96, sbs],
                                              k_rT[rb, 0, sbs])
                        nc.gpsimd.tensor_copy(kT[h][96:128, sbs],
                                              k_rT[rb, 1, sbs])

                    # q content for both heads in one psum
                    pc = ps_qk.tile([128, SB], f32, tag="pqk")
                    for lc in range(NLC):
                        nc.tensor.matmul(
                            pc[:],
                            w_qc_t[:, lc, hp * 128:(hp + 1) * 128],
                            q_dT[:, lc, sbs],
                            start=(lc == 0), stop=(lc == NLC - 1))
                    nc.vector.tensor_copy(qT[h0][0:64, sbs], pc[0:64, :])
                    nc.vector.tensor_copy(qT[h1][0:32, sbs], pc[64:96, :])
                    nc.vector.tensor_copy(qT[h1][32:64, sbs], pc[96:128, :])

                    # q rope for the pair: psum rows = [h0_lo, h1_lo,
                    # h0_hi, h1_hi] (32 each); rotate on [64,SB] slices
                    pr = ps_qk.tile([128, SB], f32, tag="pqk")
                    for lc in range(NLC):
                        nc.tensor.matmul(
                            pr[:],
                            w_qr_t[:, lc, hp * 128:(hp + 1) * 128],
                            q_dT[:, lc, sbs],
                            start=(lc == 0), stop=(lc == NLC - 1))
                    t1 = rtmp.tile([64, SB], f32, tag="t1")
                    t2 = rtmp.tile([64, SB], f32, tag="t2")
                    t3 = rtmp.tile([64, SB], f32, tag="t3")
                    t4 = rtmp.tile([64, SB], f32, tag="t4")
                    nc.vector.tensor_mul(t1[:], pr[0:64, :], cc_t[0:64, sbs])
                    nc.vector.tensor_mul(t2[:], pr[64:128, :],
                                         ss_t[64:128, sbs])
                    nc.vector.tensor_sub(t1[:], t1[:], t2[:])     # rlo
                    nc.vector.tensor_mul(t3[:], pr[64:128, :],
                                         cc_t[64:128, sbs])
                    nc.vector.tensor_mul(t4[:], pr[0:64, :], ss_t[0:64, sbs])
                    nc.vector.tensor_add(t3[:], t3[:], t4[:])     # rhi
                    # scatter rotated q rows into qT tiles (gpsimd)
                    nc.gpsimd.tensor_copy(qT[h0][64:96, sbs], t1[0:32, :])
                    nc.gpsimd.tensor_copy(qT[h1][64:96, sbs], t1[32:64, :])
                    nc.gpsimd.tensor_copy(qT[h0][96:128, sbs], t3[0:32, :])
                    nc.gpsimd.tensor_copy(qT[h1][96:128, sbs], t3[32:64, :])

                # ---- C2: causal attention, both heads interleaved -------
                nqb = s // SB
                dpq = SB // KB                      # diagonal blocks per qi

                def attn_qi(h, qi):
                    """Emit one (head, q-block) with 2-deep score lookahead."""
                    nkj = (qi + 1) * dpq
                    qs = slice(qi * SB, (qi + 1) * SB)
                    py = ps_y.tile([128, SB], f32, tag="py",
                                   name=f"py_{h}_{qi}")
                    pden = ps_b.tile([128, SB], f32, tag="pden",
                                     name=f"pden_{h}_{qi}")

                    def score(kj):
                        ps = ps_s.tile([128, SB], f32, tag="ps",
                                       name=f"ps_{h}_{qi}_{kj}")
                        nc.tensor.matmul(
                            ps[:], kT[h][:, kj * KB:(kj + 1) * KB],
                            qT[h][:, qs], start=True, stop=True)
                        return ps

                    def finish(kj, ps):
                        es = esp.tile([128, SB], mdt, tag="es",
                                      name=f"es_{h}_{qi}_{kj}")
                        nc.scalar.activation(es[:], ps[:], Exp, scale=SCALE)
                        if kj >= qi * dpq:          # diagonal block
                            nc.gpsimd.affine_select(
                                out=es[:], in_=es[:],
                                compare_op=is_ge, fill=0.0,
                                base=qi * SB - kj * KB,
                                pattern=[[1, SB]],
                                channel_multiplier=-1)
                        nc.tensor.matmul(
                            py[:], v_all[:, kj, h * HD:(h + 1) * HD], es[:],
                            start=(kj == 0), stop=(kj == nkj - 1))
                        nc.tensor.matmul(
                            pden[:], ones128[:], es[:],
                            start=(kj == 0), stop=(kj == nkj - 1))

                    pending = []
                    for kj in range(nkj):
                        pending.append((kj, score(kj)))
                        if len(pending) > 2:
                            finish(*pending.pop(0))
                    for it in pending:
                        finish(*it)

                    rec = recp.tile([128, SB], f32, tag="rec",
                                    name=f"rec_{h}_{qi}")
                    nc.vector.reciprocal(rec[:], pden[:])
                    nc.vector.tensor_mul(yT_all[:, h, qs], py[:], rec[:])

                # interleave the two heads' q-blocks for deeper parallelism
                for qi in range(nqb):
                    attn_qi(h0, qi)
                    attn_qi(h1, qi)

        # ================= PHASE D: o-projection =========================
        with ExitStack() as pd:
            wop = pd.enter_context(tc.tile_pool(name="wop", bufs=2))
            outp = pd.enter_context(tc.tile_pool(name="outp", bufs=4))
            ps_o = pd.enter_context(
                tc.tile_pool(name="ps_o", bufs=2, space="PSUM"))

            for ncol in range(H // 512):
                wo_t = wop.tile([128, HPC, 512], mdt, tag="wo")
                nc.sync.dma_start(
                    wo_t[:],
                    w_o[:, ncol * 512:(ncol + 1) * 512].rearrange(
                        "(ho p) m -> p ho m", p=128))
                for sc in range(nsc):
                    po = ps_o.tile([128, 512], f32, tag="po")
                    for hh in range(HPC):
                        nc.tensor.matmul(
                            po[:], yT_all[:, hh, sc * 128:(sc + 1) * 128],
                            wo_t[:, hh, :],
                            start=(hh == 0), stop=(hh == HPC - 1))
                    ot = outp.tile([128, 512], f32, tag="ot")
                    nc.scalar.copy(ot[:], po[:])
                    nc.sync.dma_start(
                        out[sc * 128:(sc + 1) * 128,
                            ncol * 512:(ncol + 1) * 512], ot[:])

    nc.compile()
    return nc


# ======================= host-side preparation ==========================

def _rope_tables(s):
    inv_freq = 1.0 / (ROPE_BASE ** (np.arange(0, RD, 2, dtype=np.float64) / RD))
    t = np.arange(s, dtype=np.float64)
    freqs = np.outer(t, inv_freq)                    # [s, 32]
    cc = np.tile(np.cos(freqs).T, (4, 1)).astype(np.float32)   # [128, s]
    ss = np.tile(np.sin(freqs).T, (4, 1)).astype(np.float32)
    return np.ascontiguousarray(cc), np.ascontiguousarray(ss)


def make_in_maps(hidden_states, Wkv_d, Wq_d, Wk_u, Wq_u, Wv_u, Wrk, Wrq, Wo,
                 s=S):
    f32 = np.float32
    w_down = np.ascontiguousarray(
        np.concatenate([Wkv_d, Wq_d], axis=1), dtype=f32)       # [H, 512]
    rope_cc, rope_ss = _rope_tables(s)
    Wk_u4 = Wk_u.reshape(L, NH, RD)
    Wq_u4 = Wq_u.reshape(L, NH, RD)
    Wrq4 = Wrq.reshape(L, NH, RD)
    Wv_u4 = Wv_u.reshape(L, NH, HD)
    Wrk4 = Wrk.reshape(H, NH, RD)
    Wo4 = Wo.reshape(NH, HD, H)

    def pack_lo_hi(w4, hsel, dim0):
        # [dim0, 4 heads, 64] -> cols [h0_lo..h3_lo, h0_hi..h3_hi]
        wl = w4[:, hsel, 0:RD // 2]                  # [d, 4, 32]
        wh = w4[:, hsel, RD // 2:RD]
        return np.ascontiguousarray(np.concatenate(
            [wl.reshape(dim0, HPC * 32), wh.reshape(dim0, HPC * 32)],
            axis=1), dtype=f32)                      # [d, 256]

    def pack_qr_pairs(w4, hsel):
        # per pair p: [h(2p)_lo, h(2p+1)_lo, h(2p)_hi, h(2p+1)_hi] (32 each)
        cols = []
        heads = list(range(hsel.start, hsel.stop))
        for p in range(2):
            ha, hb = heads[2 * p], heads[2 * p + 1]
            cols.extend([w4[:, ha, 0:32], w4[:, hb, 0:32],
                         w4[:, ha, 32:64], w4[:, hb, 32:64]])
        return np.ascontiguousarray(
            np.concatenate(cols, axis=1), dtype=f32)  # [L, 256]

    in_maps = []
    for c in range(8):
        b, g = divmod(c, 4)
        hsel = slice(g * HPC, (g + 1) * HPC)
        in_maps.append({
            "hs": np.ascontiguousarray(hidden_states[b, :s], dtype=f32),
            "w_down": w_down,
            # k rope: all-lo then all-hi packing (matches k_rT chunks)
            "w_rk": pack_lo_hi(Wrk4, hsel, H),
            "w_qc": np.ascontiguousarray(
                Wq_u4[:, hsel, :].reshape(L, HPC * RD), dtype=f32),
            "w_qr": pack_qr_pairs(Wrq4, hsel),
            "w_ku": np.ascontiguousarray(
                Wk_u4[:, hsel, :].reshape(L, HPC * RD), dtype=f32),
            "w_vu": np.ascontiguousarray(
                Wv_u4[:, hsel, :].reshape(L, HPC * HD), dtype=f32),
            "w_o": np.ascontiguousarray(
                Wo4[hsel].reshape(HPC * HD, H), dtype=f32),
            "rope_cc": rope_cc,
            "rope_ss": rope_ss,
        })
    return in_maps


_NC_CACHE = {}


def kernel(hidden_states, Wkv_d, Wq_d, Wk_u, Wq_u, Wv_u, Wrk, Wrq, Wo):
    from concourse.bass_utils import run_bass_kernel_spmd

    key = (S, MM_DTYPE)
    if key not in _NC_CACHE:
        _NC_CACHE[key] = build_nc(S, MM_DTYPE)
    nc = _NC_CACHE[key]

    in_maps = make_in_maps(
        np.asarray(hidden_states), np.asarray(Wkv_d), np.asarray(Wq_d),
        np.asarray(Wk_u), np.asarray(Wq_u), np.asarray(Wv_u),
        np.asarray(Wrk), np.asarray(Wrq), np.asarray(Wo))

    res = run_bass_kernel_spmd(nc, in_maps, core_ids=list(range(8)))
    parts = [r["out"] for r in res.results]
    out = np.empty((B, S, H), dtype=np.float32)
    for b in range(B):
        out[b] = parts[4 * b] + parts[4 * b + 1] + parts[4 * b + 2] + parts[4 * b + 3]
    return out


# revision 26
# speedup vs baseline: 2.1051x; 2.1051x over previous
"""Trainium2 Bass kernel for MultiHeadLatentAttention (MLA), 8-core SPMD.

Sharding: core c = (batch b=c//4, head-group g=c%4 of 4 heads).
Each core computes the full latent down-projections for its batch
(replicated across the 4 cores of that batch), head-sharded
up-projections + RoPE + causal attention, and a partial o-projection
(its heads' rows of Wo). The host sums the 4 partial outputs per batch.

Shapes (fixed): B=2, S=2048, H=2048, L=256, nh=16, hd=128, rd=64.

Device layouts (features on partitions):
  kv_dT, q_dT [L=256 -> 2x128, s]
  k_rT        [2x128, s] chunk0 = all heads' rope-lo rows (4x32),
                         chunk1 = all heads' rope-hi rows; rotated in place
  qT_h, kT_h  [128 (64 content + 64 rope), s] per head
  v_all       [128 (s%128), s//128, 4 heads * 128]  (natural v)
  yT_all      [128 (hd), 4 heads, s]

Attention is computed in scores-transposed orientation S^T[k, q] so the
probabilities feed the AV matmul directly (lhsT = v block, rhs = expS).
Softmax skips the max-subtraction (scores are tiny here: |s| < ~2). The
denominator is accumulated per k-block by an all-ones matmul into PSUM,
which also broadcasts it across partitions for the final normalize.
"""

import sys
import os

sys.path.insert(0, "/opt/trn_rl_repo")

import numpy as np

B = 2
S = 2048
H = 2048
L = 256          # latent dim (2 chunks of 128)
NH = 16          # total heads
HPC = 4          # heads per core
HD = 128         # head dim
RD = 64          # rope / content half-width
ROPE_BASE = 10000.0
SCALE = float(HD) ** -0.5

SB = 512         # s-block for projections / q-blocks in attention
KB = 128         # k-block in attention
NKC = H // 128   # 16 contraction chunks over H
NLC = L // 128   # 2 contraction chunks over L

# Matmul input dtype: "f32" (exact, 4 cyc/row) or "f32r" (1 cyc/row, ~tf32).
MM_DTYPE = os.environ.get("MLA_MM_DTYPE", "f32")


def build_nc(s=S, mm_dtype=MM_DTYPE):
    """Build the Bass module for one core. `s` can be shrunk (multiple of 512)
    for simulator testing."""
    from concourse import bacc
    import concourse.bass as bass
    import concourse.mybir as mybir
    import concourse.tile as tile
    from concourse.masks import make_identity
    from contextlib import ExitStack

    f32 = mybir.dt.float32
    f32r = mybir.dt.float32r

    # dtype for every tile/DRAM tensor that feeds a matmul: the BIR
    # verifier requires producers of fp32r-matmul operands to WRITE fp32r.
    mdt = f32r if mm_dtype == "f32r" else f32

    nsb = s // SB        # s-blocks
    nsc = s // 128       # 128-row s-chunks

    nc = bacc.Bacc(None, target_bir_lowering=False)

    hs = nc.dram_tensor("hs", [s, H], mdt, kind="ExternalInput")
    w_down = nc.dram_tensor("w_down", [H, 512], mdt, kind="ExternalInput")
    w_rk = nc.dram_tensor("w_rk", [H, HPC * RD], mdt, kind="ExternalInput")
    w_qc = nc.dram_tensor("w_qc", [L, HPC * RD], mdt, kind="ExternalInput")
    w_qr = nc.dram_tensor("w_qr", [L, HPC * RD], mdt, kind="ExternalInput")
    w_ku = nc.dram_tensor("w_ku", [L, HPC * RD], mdt, kind="ExternalInput")
    w_vu = nc.dram_tensor("w_vu", [L, HPC * HD], mdt, kind="ExternalInput")
    w_o = nc.dram_tensor("w_o", [HPC * HD, H], mdt, kind="ExternalInput")
    # cos/sin halves replicated across all four 32-partition quadrants so any
    # 32-row operand can pair with a table slice at the SAME base partition
    # (walrus: both-SBUF tensor_tensor inputs must share base partition).
    rope_cc = nc.dram_tensor("rope_cc", [128, s], f32, kind="ExternalInput")
    rope_ss = nc.dram_tensor("rope_ss", [128, s], f32, kind="ExternalInput")
    out = nc.dram_tensor("out", [s, H], f32, kind="ExternalOutput")

    Exp = mybir.ActivationFunctionType.Exp
    is_ge = mybir.AluOpType.is_ge

    with ExitStack() as top:
        tc = top.enter_context(tile.TileContext(nc))

        # ---- persistent small pools -------------------------------------
        const_pool = top.enter_context(tc.tile_pool(name="const", bufs=1))
        ident = const_pool.tile([128, 128], mdt, tag="ident")
        if mdt == f32:
            make_identity(nc, ident)
        else:
            ident_f32 = const_pool.tile([128, 128], f32, tag="ident_f32")
            make_identity(nc, ident_f32)
            nc.vector.tensor_copy(ident[:], ident_f32[:])
        ones128 = const_pool.tile([128, 128], mdt, tag="ones")
        if mdt == f32:
            nc.gpsimd.memset(ones128[:], 1.0)
        else:
            ones_f32 = const_pool.tile([128, 128], f32, tag="ones_f32")
            nc.gpsimd.memset(ones_f32[:], 1.0)
            nc.vector.tensor_copy(ones128[:], ones_f32[:])
        cc_t = const_pool.tile([128, s], f32, tag="ropec")
        ss_t = const_pool.tile([128, s], f32, tag="ropes")

        wsmall = top.enter_context(tc.tile_pool(name="wsmall", bufs=1))
        w_qc_t = wsmall.tile([128, NLC, HPC * RD], mdt, tag="wqc")
        w_qr_t = wsmall.tile([128, NLC, HPC * RD], mdt, tag="wqr")
        w_ku_t = wsmall.tile([128, NLC, HPC * RD], mdt, tag="wku")
        w_vu_t = wsmall.tile([128, NLC, HPC * HD], mdt, tag="wvu")

        # ---- latent / rope-k tensors (persist through attention) --------
        lat_pool = top.enter_context(tc.tile_pool(name="lat", bufs=1))
        kv_dT = lat_pool.tile([128, NLC, s], mdt, tag="kvd")   # [L, s]
        q_dT = lat_pool.tile([128, NLC, s], mdt, tag="qd")     # [L, s]
        k_rT = lat_pool.tile([128, NLC, s], mdt, tag="krt")    # rope k rows

        # ================= PHASE A: transpose hs + down/rope projections =
        with ExitStack() as pa:
            srcp = pa.enter_context(tc.tile_pool(name="src", bufs=4))
            hstp = pa.enter_context(tc.tile_pool(name="hst", bufs=1))
            wdp = pa.enter_context(tc.tile_pool(name="wdown", bufs=1))
            wrkp = pa.enter_context(tc.tile_pool(name="wrk", bufs=1))
            pst = pa.enter_context(tc.tile_pool(name="pst", bufs=2, space="PSUM"))
            psa = pa.enter_context(tc.tile_pool(name="psa", bufs=1, space="PSUM"))

            # per-chunk weight DMAs so the first matmuls don't wait for the
            # whole 4MB tensor
            w_down_t = wdp.tile([128, NKC, 512], mdt, tag="wd")
            w_rk_t = wrkp.tile([128, NKC, HPC * RD], mdt, tag="wr")
            wd_r = w_down.rearrange("(ko p) m -> p ko m", p=128)
            wr_r = w_rk.rearrange("(ko p) m -> p ko m", p=128)
            for kc in range(NKC):
                nc.sync.dma_start(w_down_t[:, kc, :], wd_r[:, kc, :])
                nc.sync.dma_start(w_rk_t[:, kc, :], wr_r[:, kc, :])

            names = ("kv0", "kv1", "q0", "q1", "kr0", "kr1")
            for sb in range(nsb):
                hsT = hstp.tile([128, NKC, SB], mdt, tag="hsT")
                srcs = []
                for sc in range(4):
                    srct = srcp.tile([128, H], mdt, tag="src")
                    nc.sync.dma_start(
                        srct[:],
                        hs[sb * SB + sc * 128: sb * SB + (sc + 1) * 128, :])
                    srcs.append(srct)

                # 6 accumulation banks for this s-block
                pb = {n: psa.tile([128, SB], f32, tag=f"psa_{n}",
                                  name=f"psa_{n}_{sb}") for n in names}

                for kc in range(NKC):
                    pt = pst.tile([128, SB], mdt, tag="pt")
                    for sc in range(4):
                        nc.tensor.transpose(
                            pt[:, sc * 128:(sc + 1) * 128],
                            srcs[sc][:, kc * 128:(kc + 1) * 128],
                            ident[:])
                    nc.scalar.copy(hsT[:, kc, :], pt[:])
                    rhs = hsT[:, kc, :]
                    st = kc == 0
                    sp = kc == NKC - 1
                    nc.tensor.matmul(pb["kv0"][:], w_down_t[:, kc, 0:128],
                                     rhs, start=st, stop=sp)
                    nc.tensor.matmul(pb["kv1"][:], w_down_t[:, kc, 128:256],
                                     rhs, start=st, stop=sp)
                    nc.tensor.matmul(pb["q0"][:], w_down_t[:, kc, 256:384],
                                     rhs, start=st, stop=sp)
                    nc.tensor.matmul(pb["q1"][:], w_down_t[:, kc, 384:512],
                                     rhs, start=st, stop=sp)
                    nc.tensor.matmul(pb["kr0"][:], w_rk_t[:, kc, 0:128],
                                     rhs, start=st, stop=sp)
                    nc.tensor.matmul(pb["kr1"][:], w_rk_t[:, kc, 128:256],
                                     rhs, start=st, stop=sp)

                sbs = slice(sb * SB, (sb + 1) * SB)
                nc.scalar.copy(kv_dT[:, 0, sbs], pb["kv0"][:])
                nc.scalar.copy(kv_dT[:, 1, sbs], pb["kv1"][:])
                nc.scalar.copy(q_dT[:, 0, sbs], pb["q0"][:])
                nc.scalar.copy(q_dT[:, 1, sbs], pb["q1"][:])
                nc.scalar.copy(k_rT[:, 0, sbs], pb["kr0"][:])
                nc.scalar.copy(k_rT[:, 1, sbs], pb["kr1"][:])

        # late small-weight / rope-table loads: queued behind phase A's DMAs
        nc.sync.dma_start(cc_t[:], rope_cc[:])
        nc.sync.dma_start(ss_t[:], rope_ss[:])
        nc.sync.dma_start(
            w_qc_t[:], w_qc.rearrange("(ko p) m -> p ko m", p=128))
        nc.sync.dma_start(
            w_qr_t[:], w_qr.rearrange("(ko p) m -> p ko m", p=128))
        nc.sync.dma_start(
            w_ku_t[:], w_ku.rearrange("(ko p) m -> p ko m", p=128))
        nc.sync.dma_start(
            w_vu_t[:], w_vu.rearrange("(ko p) m -> p ko m", p=128))

        # ---- rotate k_rT in place (RoPE on all 4 heads at once) ---------
        # chunk0 rows = all heads' lo dims, chunk1 = hi dims:
        #   lo' = lo*cos - hi*sin ;  hi' = hi*cos + lo*sin
        with tc.tile_pool(name="rkr", bufs=2) as rkr:
            for sb in range(nsb):
                sbs = slice(sb * SB, (sb + 1) * SB)
                r1 = rkr.tile([128, SB], f32, tag="r1")
                r2 = rkr.tile([128, SB], f32, tag="r2")
                r3 = rkr.tile([128, SB], f32, tag="r3")
                r4 = rkr.tile([128, SB], f32, tag="r4")
                lo = k_rT[:, 0, sbs].bitcast(f32)
                hi = k_rT[:, 1, sbs].bitcast(f32)
                nc.vector.tensor_mul(r1[:], lo, cc_t[:, sbs])
                nc.vector.tensor_mul(r2[:], hi, ss_t[:, sbs])
                nc.vector.tensor_mul(r3[:], hi, cc_t[:, sbs])
                nc.vector.tensor_mul(r4[:], lo, ss_t[:, sbs])
                nc.vector.tensor_sub(k_rT[:, 0, sbs], r1[:], r2[:])
                nc.vector.tensor_add(k_rT[:, 1, sbs], r3[:], r4[:])

        # ================= PHASES B-C: v, per-head q/k + attention =======
        vp = top.enter_context(tc.tile_pool(name="vp", bufs=1))
        yp = top.enter_context(tc.tile_pool(name="yp", bufs=1))
        v_all = vp.tile([128, nsc, HPC * HD], mdt, tag="vall")
        yT_all = yp.tile([128, HPC, s], mdt, tag="yT")

        with ExitStack() as pbc:
            qkp = pbc.enter_context(tc.tile_pool(name="qkp", bufs=2))
            esp = pbc.enter_context(tc.tile_pool(name="esp", bufs=4))
            recp = pbc.enter_context(tc.tile_pool(name="recp", bufs=1))
            rtmp = pbc.enter_context(tc.tile_pool(name="rtmp", bufs=1))

            # ---- PHASE B: v (natural layout), all 4 heads ----
            # scoped psum pool, closed before phase C's psum pools open
            with tc.tile_pool(name="ps_v", bufs=2, space="PSUM") as ps_v:
                for sc in range(nsc):
                    pv = ps_v.tile([128, HPC * HD], f32, tag="pv")
                    for lc in range(NLC):
                        nc.tensor.matmul(
                            pv[:], kv_dT[:, lc, sc * 128:(sc + 1) * 128],
                            w_vu_t[:, lc, :],
                            start=(lc == 0), stop=(lc == NLC - 1))
                    nc.scalar.copy(v_all[:, sc, :], pv[:])

            ps_qk = pbc.enter_context(
                tc.tile_pool(name="ps_qk", bufs=1, space="PSUM"))
            ps_s = pbc.enter_context(
                tc.tile_pool(name="ps_s", bufs=3, space="PSUM"))
            ps_y = pbc.enter_context(
                tc.tile_pool(name="ps_y", bufs=2, space="PSUM"))
            ps_b = pbc.enter_context(
                tc.tile_pool(name="ps_b", bufs=2, space="PSUM"))

            for hp in range(2):            # head pairs
                h0, h1 = 2 * hp, 2 * hp + 1
                qT = {h: qkp.tile([128, s], mdt, tag="qT", name=f"qT_{h}")
                      for h in (h0, h1)}
                kT = {h: qkp.tile([128, s], mdt, tag="kT", name=f"kT_{h}")
                      for h in (h0, h1)}

                # ---- C1: projections + rope for both heads of the pair --
                for sb in range(nsb):
                    sbs = slice(sb * SB, (sb + 1) * SB)
                    # k content for both heads in one [128, SB] psum
                    pk = ps_qk.tile([128, SB], f32, tag="pqk")
                    for lc in range(NLC):
                        nc.tensor.matmul(
                            pk[:],
                            w_ku_t[:, lc, hp * 128:(hp + 1) * 128],
                            kv_dT[:, lc, sbs],
                            start=(lc == 0), stop=(lc == NLC - 1))
                    nc.vector.tensor_copy(kT[h0][0:64, sbs], pk[0:64, :])
                    nc.vector.tensor_copy(kT[h1][0:32, sbs], pk[64:96, :])
                    nc.vector.tensor_copy(kT[h1][32:64, sbs], pk[96:128, :])
                    # k rope: scatter pre-rotated k_rT rows (gpsimd, idle)
                    for h in (h0, h1):
                        rb = slice(32 * h, 32 * h + 32)
                        nc.gpsimd.tensor_copy(kT[h][64:96, sbs],
                                              k_rT[rb, 0, sbs])
                        nc.gpsimd.tensor_copy(kT[h][96:128, sbs],
                                              k_rT[rb, 1, sbs])

                    # q content for both heads in one psum
                    pc = ps_qk.tile([128, SB], f32, tag="pqk")
                    for lc in range(NLC):
                        nc.tensor.matmul(
                            pc[:],
                            w_qc_t[:, lc, hp * 128:(hp + 1) * 128],
                            q_dT[:, lc, sbs],
                            start=(lc == 0), stop=(lc == NLC - 1))
                    nc.vector.tensor_copy(qT[h0][0:64, sbs], pc[0:64, :])
                    nc.vector.tensor_copy(qT[h1][0:32, sbs], pc[64:96, :])
                    nc.vector.tensor_copy(qT[h1][32:64, sbs], pc[96:128, :])

                    # q rope for the pair: psum rows = [h0_lo, h1_lo,
                    # h0_hi, h1_hi] (32 each); rotate on [64,SB] slices
                    pr = ps_qk.tile([128, SB], f32, tag="pqk")
                    for lc in range(NLC):
                        nc.tensor.matmul(
                            pr[:],
                            w_qr_t[:, lc, hp * 128:(hp + 1) * 128],
                            q_dT[:, lc, sbs],
                            start=(lc == 0), stop=(lc == NLC - 1))
                    t1 = rtmp.tile([64, SB], f32, tag="t1")
                    t2 = rtmp.tile([64, SB], f32, tag="t2")
                    t3 = rtmp.tile([64, SB], f32, tag="t3")
                    t4 = rtmp.tile([64, SB], f32, tag="t4")
                    nc.vector.tensor_mul(t1[:], pr[0:64, :], cc_t[0:64, sbs])
                    nc.vector.tensor_mul(t2[:], pr[64:128, :],
                                         ss_t[64:128, sbs])
                    nc.vector.tensor_sub(t1[:], t1[:], t2[:])     # rlo
                    nc.vector.tensor_mul(t3[:], pr[64:128, :],
                                         cc_t[64:128, sbs])
                    nc.vector.tensor_mul(t4[:], pr[0:64, :], ss_t[0:64, sbs])
                    nc.vector.tensor_add(t3[:], t3[:], t4[:])     # rhi
                    # scatter rotated q rows into qT tiles (gpsimd)
                    nc.gpsimd.tensor_copy(qT[h0][64:96, sbs], t1[0:32, :])
                    nc.gpsimd.tensor_copy(qT[h1][64:96, sbs], t1[32:64, :])
                    nc.gpsimd.tensor_copy(qT[h0][96:128, sbs], t3[0:32, :])
                    nc.gpsimd.tensor_copy(qT[h1][96:128, sbs], t3[32:64, :])

                # ---- C2: causal attention, both heads interleaved -------
                nqb = s // SB
                dpq = SB // KB                      # diagonal blocks per qi

                def attn_qi(h, qi):
                    """Emit one (head, q-block) with 2-deep score lookahead."""
                    nkj = (qi + 1) * dpq
                    qs = slice(qi * SB, (qi + 1) * SB)
                    py = ps_y.tile([128, SB], f32, tag="py",
                                   name=f"py_{h}_{qi}")
                    pden = ps_b.tile([128, SB], f32, tag="pden",
                                     name=f"pden_{h}_{qi}")

                    def score(kj):
                        ps = ps_s.tile([128, SB], f32, tag="ps",
                                       name=f"ps_{h}_{qi}_{kj}")
                        nc.tensor.matmul(
                            ps[:], kT[h][:, kj * KB:(kj + 1) * KB],
                            qT[h][:, qs], start=True, stop=True)
                        return ps

                    def finish(kj, ps):
                        es = esp.tile([128, SB], mdt, tag="es",
                                      name=f"es_{h}_{qi}_{kj}")
                        nc.scalar.activation(es[:], ps[:], Exp, scale=SCALE)
                        if kj >= qi * dpq:          # diagonal block
                            nc.gpsimd.affine_select(
                                out=es[:], in_=es[:],
                                compare_op=is_ge, fill=0.0,
                                base=qi * SB - kj * KB,
                                pattern=[[1, SB]],
                                channel_multiplier=-1)
                        nc.tensor.matmul(
                            py[:], v_all[:, kj, h * HD:(h + 1) * HD], es[:],
                            start=(kj == 0), stop=(kj == nkj - 1))
                        nc.tensor.matmul(
                            pden[:], ones128[:], es[:],
                            start=(kj == 0), stop=(kj == nkj - 1))

                    pending = []
                    for kj in range(nkj):
                        pending.append((kj, score(kj)))
                        if len(pending) > 2:
                            finish(*pending.pop(0))
                    for it in pending:
                        finish(*it)

                    rec = recp.tile([128, SB], f32, tag="rec",
                                    name=f"rec_{h}_{qi}")
                    nc.vector.reciprocal(rec[:], pden[:])
                    nc.vector.tensor_mul(yT_all[:, h, qs], py[:], rec[:])

                # interleave the two heads' q-blocks for deeper parallelism
                for qi in range(nqb):
                    attn_qi(h0, qi)
                    attn_qi(h1, qi)

        # ================= PHASE D: o-projection =========================
        with ExitStack() as pd:
            wop = pd.enter_context(tc.tile_pool(name="wop", bufs=2))
            outp = pd.enter_context(tc.tile_pool(name="outp", bufs=4))
            ps_o = pd.enter_context(
                tc.tile_pool(name="ps_o", bufs=2, space="PSUM"))

            for ncol in range(H // 512):
                wo_t = wop.tile([128, HPC, 512], mdt, tag="wo")
                nc.sync.dma_start(
                    wo_t[:],
                    w_o[:, ncol * 512:(ncol + 1) * 512].rearrange(
                        "(ho p) m -> p ho m", p=128))
                for sc in range(nsc):
                    po = ps_o.tile([128, 512], f32, tag="po")
                    for hh in range(HPC):
                        nc.tensor.matmul(
                            po[:], yT_all[:, hh, sc * 128:(sc + 1) * 128],
                            wo_t[:, hh, :],
                            start=(hh == 0), stop=(hh == HPC - 1))
                    ot = outp.tile([128, 512], f32, tag="ot")
                    nc.scalar.copy(ot[:], po[:])
                    nc.sync.dma_start(
                        out[sc * 128:(sc + 1) * 128,
                            ncol * 512:(ncol + 1) * 512], ot[:])

    nc.compile()
    return nc


# ======================= host-side preparation ==========================

def _rope_tables(s):
    inv_freq = 1.0 / (ROPE_BASE ** (np.arange(0, RD, 2, dtype=np.float64) / RD))
    t = np.arange(s, dtype=np.float64)
    freqs = np.outer(t, inv_freq)                    # [s, 32]
    cc = np.tile(np.cos(freqs).T, (4, 1)).astype(np.float32)   # [128, s]
    ss = np.tile(np.sin(freqs).T, (4, 1)).astype(np.float32)
    return np.ascontiguousarray(cc), np.ascontiguousarray(ss)


def make_in_maps(hidden_states, Wkv_d, Wq_d, Wk_u, Wq_u, Wv_u, Wrk, Wrq, Wo,
                 s=S):
    f32 = np.float32
    w_down = np.ascontiguousarray(
        np.concatenate([Wkv_d, Wq_d], axis=1), dtype=f32)       # [H, 512]
    rope_cc, rope_ss = _rope_tables(s)
    Wk_u4 = Wk_u.reshape(L, NH, RD)
    Wq_u4 = Wq_u.reshape(L, NH, RD)
    Wrq4 = Wrq.reshape(L, NH, RD)
    Wv_u4 = Wv_u.reshape(L, NH, HD)
    Wrk4 = Wrk.reshape(H, NH, RD)
    Wo4 = Wo.reshape(NH, HD, H)

    def pack_lo_hi(w4, hsel, dim0):
        # [dim0, 4 heads, 64] -> cols [h0_lo..h3_lo, h0_hi..h3_hi]
        wl = w4[:, hsel, 0:RD // 2]                  # [d, 4, 32]
        wh = w4[:, hsel, RD // 2:RD]
        return np.ascontiguousarray(np.concatenate(
            [wl.reshape(dim0, HPC * 32), wh.reshape(dim0, HPC * 32)],
            axis=1), dtype=f32)                      # [d, 256]

    def pack_qr_pairs(w4, hsel):
        # per pair p: [h(2p)_lo, h(2p+1)_lo, h(2p)_hi, h(2p+1)_hi] (32 each)
        cols = []
        heads = list(range(hsel.start, hsel.stop))
        for p in range(2):
            ha, hb = heads[2 * p], heads[2 * p + 1]
            cols.extend([w4[:, ha, 0:32], w4[:, hb, 0:32],
                         w4[:, ha, 32:64], w4[:, hb, 32:64]])
        return np.ascontiguousarray(
            np.concatenate(cols, axis=1), dtype=f32)  # [L, 256]

    in_maps = []
    for c in range(8):
        b, g = divmod(c, 4)
        hsel = slice(g * HPC, (g + 1) * HPC)
        in_maps.append({
            "hs": np.ascontiguousarray(hidden_states[b, :s], dtype=f32),
            "w_down": w_down,
            # k rope: all-lo then all-hi packing (matches k_rT chunks)
            "w_rk": pack_lo_hi(Wrk4, hsel, H),
            "w_qc": np.ascontiguousarray(
                Wq_u4[:, hsel, :].reshape(L, HPC * RD), dtype=f32),
            "w_qr": pack_qr_pairs(Wrq4, hsel),
            "w_ku": np.ascontiguousarray(
                Wk_u4[:, hsel, :].reshape(L, HPC * RD), dtype=f32),
            "w_vu": np.ascontiguousarray(
                Wv_u4[:, hsel, :].reshape(L, HPC * HD), dtype=f32),
            "w_o": np.ascontiguousarray(
                Wo4[hsel].reshape(HPC * HD, H), dtype=f32),
            "rope_cc": rope_cc,
            "rope_ss": rope_ss,
        })
    return in_maps


_NC_CACHE = {}


def kernel(hidden_states, Wkv_d, Wq_d, Wk_u, Wq_u, Wv_u, Wrk, Wrq, Wo):
    from concourse.bass_utils import run_bass_kernel_spmd

    key = (S, MM_DTYPE)
    if key not in _NC_CACHE:
        _NC_CACHE[key] = build_nc(S, MM_DTYPE)
    nc = _NC_CACHE[key]

    in_maps = make_in_maps(
        np.asarray(hidden_states), np.asarray(Wkv_d), np.asarray(Wq_d),
        np.asarray(Wk_u), np.asarray(Wq_u), np.asarray(Wv_u),
        np.asarray(Wrk), np.asarray(Wrq), np.asarray(Wo))

    res = run_bass_kernel_spmd(nc, in_maps, core_ids=list(range(8)))
    parts = [r["out"] for r in res.results]
    out = np.empty((B, S, H), dtype=np.float32)
    for b in range(B):
        out[b] = parts[4 * b] + parts[4 * b + 1] + parts[4 * b + 2] + parts[4 * b + 3]
    return out
